# revision 10
# baseline (speedup 1.0000x reference)
"""AWQ int4 linear + fused LoRA on 8 Trainium2 NeuronCores.

Computes out = x @ dequant(qweight, qzeros, scales) + (x @ lora_a) @ lora_b
with tensor-parallel column sharding over N (no collectives needed).

Per-core device kernel:
  Phase A: dequantize the per-core weight shard W = (q - z) * s into SBUF
    (bf16), folding in the LoRA product A@B so the main GEMM computes
    everything at once.  The shard arrives host-transposed ([N, K]) so the
    group-wise (z, s) land on the partition axis and dequant is a single
    fused tensor_scalar op per (n-tile, group); PE transposes flip tiles
    into the [K, N] matmul layout while an A@B matmul per tile accumulates
    the LoRA term; one scalar_tensor_tensor evict merges both into SBUF.
  Phase B: dense bf16 GEMM x^T-tiles (k-major, cast to bf16 in-flight by
    SWDGE DMA) against the resident W, accumulating fp32 in PSUM.
"""

import sys

if "/opt/trn_rl_repo" not in sys.path:
    sys.path.insert(0, "/opt/trn_rl_repo")

import numpy as np

P = 128
N_CORES = 8
T_FULL, K_FULL, N_FULL = 8192, 4096, 11008
R_FULL = 64
NSH = N_FULL // N_CORES  # 1376 columns per core
NP_FULL = 1408  # padded to 11 * 128


def _n_slices(NP, max_free=512):
    out = []
    off = 0
    while off < NP:
        ns = min(max_free, NP - off)
        out.append((off, ns))
        off += ns
    return out


def _patched_tile_context(tile_mod, nc):
    """TileContext whose tail drain keeps <=1 sem wait per SP instruction
    (this walrus build rejects >2 sync waits on a Drain)."""
    from bass_rust import ScopedClock, SyncInfo

    class TileContextPatched(tile_mod.TileContext):
        def _drain_and_barrier(self, tick_clock, wait_clock):
            drain_inst = self.nc.sync.drain()
            wait_clock.add_sem_waits(
                drain_inst.ins, ScopedClock({None: tick_clock.global_clock})
            )
            waits = list(drain_inst.ins.sync_info.on_wait)
            if len(waits) > 1:
                drain_inst.ins.sync_info.on_wait = waits[:1]
                for w in waits[1:]:
                    nop = self.nc.sync.nop()
                    nop.ins.sync_info = SyncInfo(on_wait=[w], on_update=[])

            self.nc.all_engine_barrier()
            assert self.sems is not None
            popped = self.nc._tile_sem_poison_stack.pop()
            assert popped is self._sem_poison
            self.nc.clear_and_free_semaphores(list(self.sems.allocated().values()))
            self.nc.all_engine_barrier()

    return TileContextPatched(nc)


def _split_multi_waits(nc, max_waits=1):
    """This walrus build rejects instructions carrying more than ~1-2 sem
    waits ('Too many sync wait commands').  Move extra waits onto standalone
    EventSemaphore instructions inserted just before, on the same engine —
    engines execute their stream in order, so this is semantically identical.
    """
    from concourse import mybir

    n_split = 0
    for f in nc.m.functions:
        for bb in f.blocks:
            insts = list(bb.instructions)
            out, changed = [], False
            for inst in insts:
                si = inst.sync_info
                if si is not None and len(si.on_wait) > max_waits:
                    waits = list(si.on_wait)
                    for w in waits[:-max_waits]:
                        n_split += 1
                        nop = mybir.InstEventSemaphore(
                            name=f"{inst.name}-ws{n_split}", ins=[], outs=[])
                        nop.engine = inst.engine
                        nop.sync_info = mybir.SyncInfo(on_wait=[w], on_update=[])
                        out.append(nop)
                    si.on_wait = waits[-max_waits:]
                    changed = True
                out.append(inst)
            if changed:
                bb.instructions = out
    return n_split


def build_bass(T=T_FULL, K=K_FULL, NP=NP_FULL, R=R_FULL, TSUP=256,
               num_devices=N_CORES, split_waits=True):
    """Build the per-core Bass program (SPMD: all cores run this)."""
    import concourse.bass as bass
    import concourse.tile as tile
    from concourse import mybir
    from concourse.masks import make_identity

    NG = K // P  # k-tiles; == quant groups (group size 128)
    NT = NP // P  # n-tiles
    assert T % TSUP == 0 and TSUP % P == 0
    f32, bf16, i32 = mybir.dt.float32, mybir.dt.bfloat16, mybir.dt.int32
    Alu = mybir.AluOpType

    nc = bass.Bass("TRN2", target_bir_lowering=False, debug=False,
                   num_devices=num_devices)
    xt_d = nc.dram_tensor("xt", [K, T], f32, kind="ExternalInput")
    qt_d = nc.dram_tensor("qt", [NP, K], i32, kind="ExternalInput")
    zt_d = nc.dram_tensor("zt", [NP, NG], i32, kind="ExternalInput")
    st_d = nc.dram_tensor("st", [NP, NG], f32, kind="ExternalInput")
    at_d = nc.dram_tensor("at", [R, K], f32, kind="ExternalInput")
    b_d = nc.dram_tensor("b", [R, NP], f32, kind="ExternalInput")
    out_d = nc.dram_tensor("out", [T, NP], f32, kind="ExternalOutput")

    slices = _n_slices(NP)

    from contextlib import ExitStack

    tc = _patched_tile_context(tile, nc)
    with tc, ExitStack() as ctx:
        const = ctx.enter_context(tc.tile_pool(name="const", bufs=1))
        ident = const.tile([P, P], f32, name="ident")
        make_identity(nc, ident[:])
        at_sb = const.tile([R, K], bf16, name="at_sb")
        nc.gpsimd.dma_start(at_sb[:], at_d.ap())
        b_sb = const.tile([R, NP], bf16, name="b_sb")
        nc.gpsimd.dma_start(b_sb[:], b_d.ap())

        wpool = ctx.enter_context(tc.tile_pool(name="wpool", bufs=1))
        W_sb = wpool.tile([P, NG, NP], bf16, name="W_sb")

        # ---- Phase A: dequant + LoRA fold ----
        with tc.tile_pool(name="deq", bufs=2) as deq, \
             tc.tile_pool(name="deq_ps", bufs=2, space="PSUM") as deq_ps:
            for i in range(NT):
                qt_t = deq.tile([P, K], i32, name="qt_t")
                nc.sync.dma_start(qt_t[:], qt_d.ap()[i * P:(i + 1) * P, :])
                zt_t = deq.tile([P, NG], i32, name="zt_t")
                nc.sync.dma_start(zt_t[:], zt_d.ap()[i * P:(i + 1) * P, :])
                st_t = deq.tile([P, NG], f32, name="st_t")
                nc.sync.dma_start(st_t[:], st_d.ap()[i * P:(i + 1) * P, :])
                ztf = deq.tile([P, NG], f32, name="ztf")
                nc.vector.tensor_copy(ztf[:], zt_t[:])

                wt_t = deq.tile([P, K], f32, name="wt_t")
                for g in range(NG):
                    nc.vector.tensor_scalar(
                        out=wt_t[:, g * P:(g + 1) * P],
                        in0=qt_t[:, g * P:(g + 1) * P],
                        scalar1=ztf[:, g:g + 1],
                        scalar2=st_t[:, g:g + 1],
                        op0=Alu.subtract,
                        op1=Alu.mult,
                    )
                for j in range(NG):
                    ps_w = deq_ps.tile([P, P], f32, name="ps_w")
                    nc.tensor.transpose(ps_w[:], wt_t[:, j * P:(j + 1) * P],
                                        ident[:])
                    # accumulate the LoRA A@B chunk into the same psum tile
                    nc.tensor.matmul(
                        ps_w[:],
                        lhsT=at_sb[:, j * P:(j + 1) * P],
                        rhs=b_sb[:, i * P:(i + 1) * P],
                        start=False, stop=True,
                        skip_group_check=True,
                    )
                    nc.vector.tensor_copy(
                        W_sb[:, j][:, i * P:(i + 1) * P], ps_w[:]
                    )

        # ---- Phase B: main GEMM ----
        with tc.tile_pool(name="xb", bufs=2) as xb, \
             tc.tile_pool(name="ob", bufs=2) as ob, \
             tc.tile_pool(name="mm_ps", bufs=2 * len(slices), space="PSUM") as mm_ps:
            xt_r = xt_d.ap().rearrange("(j p) t -> p j t", p=P)
            for sidx in range(T // TSUP):
                t0 = sidx * TSUP
                x_t = xb.tile([P, NG, TSUP], bf16, name="x_t")
                nc.gpsimd.dma_start(x_t[:], xt_r[:, :, t0:t0 + TSUP])
                for tsub in range(TSUP // P):
                    psums = []
                    for (off, ns) in slices:
                        pt = mm_ps.tile([P, 512], f32, name="mm_ps")
                        psums.append(pt)
                    for j in range(NG):
                        lhsT = x_t[:, j][:, tsub * P:(tsub + 1) * P]
                        for si, (off, ns) in enumerate(slices):
                            nc.tensor.matmul(
                                psums[si][:, :ns],
                                lhsT=lhsT,
                                rhs=W_sb[:, j][:, off:off + ns],
                                start=(j == 0),
                                stop=(j == NG - 1),
                            )
                    out_t = ob.tile([P, NP], f32, name="out_t")
                    for si, (off, ns) in enumerate(slices):
                        nc.vector.tensor_copy(out_t[:, off:off + ns],
                                              psums[si][:, :ns])
                    nc.sync.dma_start(
                        out_d.ap()[t0 + tsub * P:t0 + (tsub + 1) * P, :],
                        out_t[:],
                    )
    if split_waits:
        _split_multi_waits(nc)
    return nc


def _marshal_inputs(x, scales, lora_a, lora_b, qweight, qzeros,
                    n_cores=N_CORES, NP=NP_FULL):
    """Host-side sharding + layout prep (pure data movement / padding)."""
    x = np.asarray(x, dtype=np.float32)
    scales = np.asarray(scales, dtype=np.float32)
    lora_a = np.asarray(lora_a, dtype=np.float32)
    lora_b = np.asarray(lora_b, dtype=np.float32)
    qweight = np.asarray(qweight, dtype=np.int32)
    qzeros = np.asarray(qzeros, dtype=np.int32)

    K, N = qweight.shape
    NG = scales.shape[0]
    nsh = N // n_cores

    xt = np.ascontiguousarray(x.T)          # [K, T]
    at = np.ascontiguousarray(lora_a.T)     # [R, K]
    qT = np.ascontiguousarray(qweight.T)    # [N, K]
    zT = np.ascontiguousarray(qzeros.T)     # [N, NG]
    sT = np.ascontiguousarray(scales.T)     # [N, NG]

    in_maps = []
    for c in range(n_cores):
        lo, hi = c * nsh, (c + 1) * nsh
        qt = np.zeros((NP, K), np.int32)
        qt[:nsh] = qT[lo:hi]
        zt = np.zeros((NP, NG), np.int32)
        zt[:nsh] = zT[lo:hi]
        st = np.zeros((NP, NG), np.float32)
        st[:nsh] = sT[lo:hi]
        b = np.zeros((lora_b.shape[0], NP), np.float32)
        b[:, :nsh] = lora_b[:, lo:hi]
        in_maps.append({"xt": xt, "qt": qt, "zt": zt, "st": st, "at": at, "b": b})
    return in_maps, nsh


_NC_CACHE = {}


def kernel(x, scales, lora_a, lora_b, qweight, qzeros):
    from concourse.bass_utils import run_bass_kernel_spmd

    in_maps, nsh = _marshal_inputs(x, scales, lora_a, lora_b, qweight, qzeros)
    key = "full"
    if key not in _NC_CACHE:
        _NC_CACHE[key] = build_bass()
    nc = _NC_CACHE[key]
    res = run_bass_kernel_spmd(nc, in_maps, core_ids=list(range(N_CORES)),
                               trace=False)
    outs = [res.results[c]["out"][:, :nsh] for c in range(N_CORES)]
    return np.ascontiguousarray(np.concatenate(outs, axis=1))
